# revision 8
# baseline (speedup 1.0000x reference)
"""GAT-RW GNN message-passing kernel for 8 trn2 NeuronCores (Bass/Tile).

Strategy:
- Shard nodes across 8 cores (6250 real + 22 pad = 6272/core, 50176 padded).
- Per layer: each core builds its table shard [6272, 144] = [h(128) | ar(1) |
  pad(15)] from its own inputs, AllGather -> full table [50176, 144] in each
  core's DRAM, then gathers its 100352 walk endpoints (4 hops x 6272 x 16)
  row-by-row-per-partition via indirect DMA ([128,1] offsets -> [128,144]
  rows, the only gather primitive that works on this runtime).
- Node-major gather layout [128 nodes, 16 samples, 144]: softmax over samples
  is pure per-partition free-dim math; weighted mean = one fat strided DVE
  multiply + one strided reduce per (tile, hop).
- Head: PE transpose + lout matmul + log_softmax per 128-node tile.
"""
import sys
sys.path.insert(0, "/opt/trn_rl_repo")
sys.path.insert(0, "/root/.axon_site/_ro/trn_rl_repo")
import numpy as np

import bass_rust
import concourse.bass as bass
import concourse.mybir as mybir
import concourse.tile as tile
from concourse.bass_utils import run_bass_kernel_spmd

# problem constants (hardcoded per spec)
N, F_IN, H, C, K, NLAYER, RWS = 50000, 256, 128, 64, 4, 2, 16
NCORES = 8
NC_ = 6272                # padded nodes per core (49 * 128)
NPAD = NC_ * NCORES       # 50176
NT = NC_ // 128           # 49 node tiles per core
RW = 144                  # table row floats: h(128) | ar(1) | pad(15) -> 576B
P = 128
F32 = mybir.dt.float32
I32 = mybir.dt.int32


def _split_multi_waits(nc):
    """This walrus accepts at most one sem wait per instruction; Tile emits
    multi-wait instructions. Move extra waits onto same-engine carrier NOPs."""
    for f in nc.m.functions:
        for bb in f.blocks:
            insts = bb.instructions
            out = []
            changed = False
            for ins in insts:
                si = ins.sync_info
                ow = list(si.on_wait) if si is not None and si.on_wait else []
                keep = 0 if type(ins).__name__ == "InstDrain" else 1
                if len(ow) > keep:
                    move = ow[: len(ow) - keep]
                    stay = ow[len(ow) - keep:]
                    for w in move:
                        nop = mybir.InstNoOp(
                            name=nc.get_next_instruction_name(), ins=[], outs=[]
                        )
                        nop.engine = ins.engine
                        nop.sync_info = bass_rust.SyncInfo(on_wait=[w], on_update=[])
                        out.append(nop)
                    si.on_wait = stay
                    changed = True
                out.append(ins)
            if changed:
                bb.instructions = out


def _build_nc(att_np, phases=5):
    """Build the SPMD program (identical on all cores). att values are baked
    as immediates (available at kernel build time)."""
    nc = bass.Bass()

    # per-core external inputs
    xTa = nc.dram_tensor("xTa", [P, NC_], F32, kind="ExternalInput")   # xT rows 0:128
    xTb = nc.dram_tensor("xTb", [P, NC_], F32, kind="ExternalInput")   # xT rows 128:256
    ends_g = nc.dram_tensor("ends_g", [NLAYER, P, K * NT * 16], I32, kind="ExternalInput")
    w0t = nc.dram_tensor("w0t", [2, P, H], F32, kind="ExternalInput")  # lin0_w.T k-chunks
    w1t = nc.dram_tensor("w1t", [P, H], F32, kind="ExternalInput")     # lin1_w.T
    loutt = nc.dram_tensor("loutt", [P, C], F32, kind="ExternalInput") # lout_w.T
    b0r = nc.dram_tensor("b0r", [P, H], F32, kind="ExternalInput")     # bias rows replicated
    b1r = nc.dram_tensor("b1r", [P, H], F32, kind="ExternalInput")
    bor = nc.dram_tensor("bor", [P, C], F32, kind="ExternalInput")
    attlr = nc.dram_tensor("attlr", [NLAYER, P, H], F32, kind="ExternalInput")
    ident_in = nc.dram_tensor("ident_in", [P, P], F32, kind="ExternalInput")
    attrr = nc.dram_tensor("attrr", [NLAYER, P, H], F32, kind="ExternalInput")

    out_sh = nc.dram_tensor("out_sh", [NC_, C], F32, kind="ExternalOutput")

    # internal DRAM
    t_shard = [nc.dram_tensor(f"t{l}_shard", [NC_, RW], F32) for l in range(NLAYER)]
    t_full = [
        nc.dram_tensor(f"t{l}_full", [NPAD, RW], F32, addr_space="Shared")
        for l in range(NLAYER)
    ]

    core_ids = list(range(NCORES))

    with tile.TileContext(nc) as tc:
        with (
            tc.tile_pool(name="const", bufs=1) as cpool,
            tc.tile_pool(name="bld", bufs=3) as bpool,
            tc.tile_pool(name="gat", bufs=3) as gpool,
            tc.tile_pool(name="mac", bufs=2) as mpool,
            tc.tile_pool(name="sm", bufs=4) as spool,
            tc.tile_pool(name="big", bufs=1) as bigpool,
            tc.tile_pool(name="ps", bufs=2, space="PSUM") as pspool,
            tc.tile_pool(name="ps2", bufs=2, space="PSUM") as ps2pool,
        ):
            # constants in SBUF
            ident = cpool.tile([P, P], F32)
            nc.sync.dma_start(out=ident[:], in_=ident_in[:])
            w0a_sb = cpool.tile([P, H], F32)
            nc.sync.dma_start(out=w0a_sb[:], in_=w0t[0])
            w0b_sb = cpool.tile([P, H], F32)
            nc.sync.dma_start(out=w0b_sb[:], in_=w0t[1])
            w1t_sb = cpool.tile([P, H], F32)
            nc.sync.dma_start(out=w1t_sb[:], in_=w1t[:])
            loutt_sb = cpool.tile([P, C], F32)
            nc.sync.dma_start(out=loutt_sb[:], in_=loutt[:])
            b0_sb = cpool.tile([P, H], F32)
            nc.sync.dma_start(out=b0_sb[:], in_=b0r[:])
            b1_sb = cpool.tile([P, H], F32)
            nc.sync.dma_start(out=b1_sb[:], in_=b1r[:])
            bo_sb = cpool.tile([P, C], F32)
            nc.sync.dma_start(out=bo_sb[:], in_=bor[:])
            attl_sb = [cpool.tile([P, H], F32, tag=f"attl{l}", name=f"attl{l}") for l in range(NLAYER)]
            attr_sb = [cpool.tile([P, H], F32, tag=f"attr{l}", name=f"attr{l}") for l in range(NLAYER)]
            for l in range(NLAYER):
                nc.sync.dma_start(out=attl_sb[l][:], in_=attlr[l])
                nc.sync.dma_start(out=attr_sb[l][:], in_=attrr[l])
            al_own = cpool.tile([P, NT], F32)
            agg_all = bigpool.tile([P, NC_], F32)   # agg tiles, col-blocked
            ends_sb = cpool.tile([P, K * NT * 16], I32)

            xa_sb = cpool.tile([P, NC_], F32)
            nc.sync.dma_start(out=xa_sb[:], in_=xTa[:])
            xb_sb = cpool.tile([P, NC_], F32)
            nc.sync.dma_start(out=xb_sb[:], in_=xTb[:])

            def build_tile_l0(t):
                ps = pspool.tile([P, H], F32, tag="bps")
                nc.tensor.matmul(out=ps[:], lhsT=xa_sb[:, t * P:(t + 1) * P],
                                 rhs=w0a_sb[:], start=True, stop=False)
                nc.tensor.matmul(out=ps[:], lhsT=xb_sb[:, t * P:(t + 1) * P],
                                 rhs=w0b_sb[:], start=False, stop=True)
                tb = bpool.tile([P, RW], F32, tag="tbl")
                nc.vector.tensor_tensor(out=tb[:, 0:H], in0=ps[:], in1=b0_sb[:],
                                        op=mybir.AluOpType.add)
                nc.vector.memset(tb[:, H:RW], 0.0)
                tr = mpool.tile([P, H], F32, tag="trash")
                nc.vector.tensor_tensor(out=tr[:], in0=tb[:, 0:H],
                                        in1=attr_sb[0][:], op=mybir.AluOpType.mult)
                nc.vector.tensor_reduce(out=tb[:, H:H + 1], in_=tr[:],
                                        axis=mybir.AxisListType.X,
                                        op=mybir.AluOpType.add)
                nc.vector.tensor_tensor(out=tr[:], in0=tb[:, 0:H],
                                        in1=attl_sb[0][:], op=mybir.AluOpType.mult)
                nc.vector.tensor_reduce(out=al_own[:, t:t + 1], in_=tr[:],
                                        axis=mybir.AxisListType.X,
                                        op=mybir.AluOpType.add)
                nc.sync.dma_start(out=t_shard[0][t * P:(t + 1) * P, :], in_=tb[:])

            def build_tile_l1(t):
                # transpose agg tile -> lhsT
                pst = ps2pool.tile([P, P], F32, tag="tps")
                nc.tensor.transpose(out=pst[:], in_=agg_all[:, t * P:(t + 1) * P],
                                    identity=ident[:])
                aggT = bpool.tile([P, P], F32, tag="aggT")
                nc.vector.tensor_copy(out=aggT[:], in_=pst[:])
                ps = pspool.tile([P, H], F32, tag="bps")
                nc.tensor.matmul(out=ps[:], lhsT=aggT[:], rhs=w1t_sb[:],
                                 start=True, stop=True)
                tb = bpool.tile([P, RW], F32, tag="tbl")
                nc.vector.tensor_tensor(out=tb[:, 0:H], in0=ps[:], in1=b1_sb[:],
                                        op=mybir.AluOpType.add)
                nc.vector.memset(tb[:, H:RW], 0.0)
                tr = mpool.tile([P, H], F32, tag="trash")
                nc.vector.tensor_tensor(out=tr[:], in0=tb[:, 0:H],
                                        in1=attr_sb[1][:], op=mybir.AluOpType.mult)
                nc.vector.tensor_reduce(out=tb[:, H:H + 1], in_=tr[:],
                                        axis=mybir.AxisListType.X,
                                        op=mybir.AluOpType.add)
                nc.vector.tensor_tensor(out=tr[:], in0=tb[:, 0:H],
                                        in1=attl_sb[1][:], op=mybir.AluOpType.mult)
                nc.vector.tensor_reduce(out=al_own[:, t:t + 1], in_=tr[:],
                                        axis=mybir.AxisListType.X,
                                        op=mybir.AluOpType.add)
                nc.sync.dma_start(out=t_shard[1][t * P:(t + 1) * P, :], in_=tb[:])

            def aggregate_layer(l):
                att0 = float(att_np[l, 0])
                nc.sync.dma_start(out=ends_sb[:], in_=ends_g[l])
                for t in range(NT):
                    agg_sl = agg_all[:, t * P:(t + 1) * P]
                    hown = gpool.tile([P, H], F32, tag="hown")
                    nc.sync.dma_start(out=hown[:],
                                      in_=t_shard[l][t * P:(t + 1) * P, 0:H])
                    nc.scalar.activation(out=agg_sl, in_=hown[:],
                                         func=mybir.ActivationFunctionType.Copy,
                                         scale=att0)
                    for k in range(K):
                        sc = float(att_np[l, k + 1]) / RWS
                        G = gpool.tile([P, RWS, RW], F32, tag="G")
                        base = k * NT * 16 + t * 16
                        for s in range(RWS):
                            nc.gpsimd.indirect_dma_start(
                                out=G[:, s, :], out_offset=None,
                                in_=t_full[l][:],
                                in_offset=bass.IndirectOffsetOnAxis(
                                    ap=ends_sb[:, base + s:base + s + 1], axis=0))
                        ar_g = G[:, :, H]                      # [128, 16] strided
                        lg = spool.tile([P, RWS], F32, tag="lg")
                        nc.vector.tensor_scalar(
                            out=lg[:], in0=ar_g, scalar1=al_own[:, t:t + 1],
                            scalar2=None, op0=mybir.AluOpType.add)
                        lr = spool.tile([P, RWS], F32, tag="lr")
                        nc.scalar.activation(out=lr[:], in_=lg[:],
                                             func=mybir.ActivationFunctionType.Lrelu,
                                             alpha=0.2)
                        ex = spool.tile([P, RWS], F32, tag="ex")
                        nc.scalar.activation(out=ex[:], in_=lr[:],
                                             func=mybir.ActivationFunctionType.Exp)
                        sm = spool.tile([P, 1], F32, tag="sm")
                        nc.vector.tensor_reduce(out=sm[:], in_=ex[:],
                                                axis=mybir.AxisListType.X,
                                                op=mybir.AluOpType.add)
                        rc = spool.tile([P, 1], F32, tag="rc")
                        nc.vector.reciprocal(out=rc[:], in_=sm[:])
                        wt = spool.tile([P, RWS], F32, tag="wt")
                        nc.vector.tensor_scalar(
                            out=wt[:], in0=ex[:], scalar1=rc[:, 0:1], scalar2=sc,
                            op0=mybir.AluOpType.mult, op1=mybir.AluOpType.mult)
                        mt = mpool.tile([P, RWS, H], F32, tag="mt")
                        nc.vector.tensor_tensor(
                            out=mt[:], in0=G[:, :, 0:H],
                            in1=wt[:, :, None].broadcast_to([P, RWS, H]),
                            op=mybir.AluOpType.mult)
                        ht = mpool.tile([P, H], F32, tag="ht")
                        nc.vector.tensor_reduce(
                            out=ht[:], in_=mt[:].rearrange("p c f -> p f c"),
                            axis=mybir.AxisListType.X, op=mybir.AluOpType.add)
                        nc.vector.tensor_tensor(out=agg_sl, in0=agg_sl, in1=ht[:],
                                                op=mybir.AluOpType.add)

            def head_tile(t):
                pst = ps2pool.tile([P, P], F32, tag="tps")
                nc.tensor.transpose(out=pst[:], in_=agg_all[:, t * P:(t + 1) * P],
                                    identity=ident[:])
                aggT = bpool.tile([P, P], F32, tag="aggT")
                nc.vector.tensor_copy(out=aggT[:], in_=pst[:])
                ps = pspool.tile([P, C], F32, tag="hps")
                nc.tensor.matmul(out=ps[:], lhsT=aggT[:], rhs=loutt_sb[:],
                                 start=True, stop=True)
                lgt = bpool.tile([P, C], F32, tag="lgt")
                nc.vector.tensor_tensor(out=lgt[:], in0=ps[:], in1=bo_sb[:],
                                        op=mybir.AluOpType.add)
                mx = spool.tile([P, 1], F32, tag="mx")
                nc.vector.tensor_reduce(out=mx[:], in_=lgt[:],
                                        axis=mybir.AxisListType.X,
                                        op=mybir.AluOpType.max)
                mneg = spool.tile([P, 1], F32, tag="mneg")
                nc.vector.tensor_scalar(out=mneg[:], in0=mx[:], scalar1=-1.0,
                                        scalar2=None, op0=mybir.AluOpType.mult)
                ev = bpool.tile([P, C], F32, tag="ev")
                se = spool.tile([P, 1], F32, tag="se")
                nc.scalar.activation(out=ev[:], in_=lgt[:],
                                     func=mybir.ActivationFunctionType.Exp,
                                     bias=mneg[:, 0:1], accum_out=se[:])
                lnse = spool.tile([P, 1], F32, tag="lnse")
                nc.scalar.activation(out=lnse[:], in_=se[:],
                                     func=mybir.ActivationFunctionType.Ln)
                tot = spool.tile([P, 1], F32, tag="tot")
                nc.vector.tensor_tensor(out=tot[:], in0=mx[:], in1=lnse[:],
                                        op=mybir.AluOpType.add)
                ot = bpool.tile([P, C], F32, tag="ot")
                nc.vector.tensor_scalar(out=ot[:], in0=lgt[:], scalar1=tot[:, 0:1],
                                        scalar2=None,
                                        op0=mybir.AluOpType.subtract)
                nc.sync.dma_start(out=out_sh[t * P:(t + 1) * P, :], in_=ot[:])

            # ---- phases ----
            for t in range(NT):
                build_tile_l0(t)
            if phases >= 1:
                tc.strict_bb_all_engine_barrier()
                nc.gpsimd.collective_compute(
                    "AllGather", mybir.AluOpType.bypass,
                    replica_groups=[core_ids],
                    ins=[t_shard[0][:]], outs=[t_full[0][:]])
                tc.strict_bb_all_engine_barrier()
            if phases >= 2:
                aggregate_layer(0)
            if phases >= 3:
                for t in range(NT):
                    build_tile_l1(t)
                tc.strict_bb_all_engine_barrier()
                nc.gpsimd.collective_compute(
                    "AllGather", mybir.AluOpType.bypass,
                    replica_groups=[core_ids],
                    ins=[t_shard[1][:]], outs=[t_full[1][:]])
                tc.strict_bb_all_engine_barrier()
            if phases >= 4:
                aggregate_layer(1)
            if phases >= 5:
                for t in range(NT):
                    head_tile(t)
            if phases < 5:
                z = bpool.tile([P, C], F32, tag="z")
                nc.vector.memset(z[:], 0.0)
                for t in range(NT):
                    nc.sync.dma_start(out=out_sh[t * P:(t + 1) * P, :], in_=z[:])

    from concourse.library_overlay import lower_extended_insts
    lower_extended_insts(nc)
    _split_multi_waits(nc)
    return nc


def _prep_inputs(x, ends, lin0_w, lin0_b, lin1_w, lin1_b, lout_w, lout_b,
                 attl_w, attl_b, attr_w, attr_b, att):
    ends = np.asarray(ends).astype(np.int32)
    x = np.asarray(x, dtype=np.float32)
    # pad + transpose x
    xp = np.zeros((NPAD, F_IN), np.float32)
    # node remap: global g -> padded core-local layout
    # core = g // 6250, local = g % 6250, pid = core*6272 + local
    g = np.arange(N)
    pid_of = (g // 6250) * NC_ + (g % 6250)
    xp[pid_of] = x
    xT = np.ascontiguousarray(xp.T)                      # [256, 50176]

    # remap endpoint ids to padded space
    er = ends.reshape(NLAYER, K, N, RWS)
    er_pid = pid_of[er]                                  # [L, K, N, RWS]

    in_maps = []
    for c in range(NCORES):
        lo, hi = c * 6250, (c + 1) * 6250
        # per-core ends_g [L, 128, K*NT*16]: node tile t partition p ->
        # node local = t*128+p (pad nodes -> endpoint 0)
        eg = np.zeros((NLAYER, P, K * NT * 16), np.int32)
        for l in range(NLAYER):
            src = er_pid[l, :, lo:hi, :]                 # [K, 6250, 16]
            full = np.zeros((K, NC_, RWS), np.int32)
            full[:, :6250, :] = src
            # layout col = k*NT*16 + t*16 + s ; partition = p
            v = full.reshape(K, NT, P, RWS)
            eg[l] = v.transpose(2, 0, 1, 3).reshape(P, K * NT * RWS)
        in_maps.append({
            "xTa": np.ascontiguousarray(xT[:128, c * NC_:(c + 1) * NC_]),
            "xTb": np.ascontiguousarray(xT[128:, c * NC_:(c + 1) * NC_]),
            "ends_g": eg,
            "w0t": np.ascontiguousarray(
                lin0_w.T.astype(np.float32).reshape(2, P, H)),
            "w1t": np.ascontiguousarray(lin1_w.T.astype(np.float32)),
            "loutt": np.ascontiguousarray(lout_w.T.astype(np.float32)),
            "b0r": np.tile(np.asarray(lin0_b, np.float32), (P, 1)),
            "b1r": np.tile(np.asarray(lin1_b, np.float32), (P, 1)),
            "bor": np.tile(np.asarray(lout_b, np.float32), (P, 1)),
            "ident_in": np.eye(P, dtype=np.float32),
            "attlr": np.stack([np.tile(np.asarray(attl_w, np.float32)[l], (P, 1))
                               for l in range(NLAYER)]),
            "attrr": np.stack([np.tile(np.asarray(attr_w, np.float32)[l], (P, 1))
                               for l in range(NLAYER)]),
        })
    return in_maps, pid_of


_CACHE = {}


def kernel(**inputs):
    att = np.asarray(inputs["att"], dtype=np.float32)
    key = att.tobytes()
    if key not in _CACHE:
        _CACHE[key] = _build_nc(att)
    nc = _CACHE[key]
    in_maps, pid_of = _prep_inputs(**inputs)
    res = run_bass_kernel_spmd(nc, in_maps, list(range(NCORES)))
    out = np.zeros((N, C), np.float32)
    for c in range(NCORES):
        out[c * 6250:(c + 1) * 6250] = res.results[c]["out_sh"][:6250]
    return out


# revision 9
# speedup vs baseline: 1.0213x; 1.0213x over previous
"""GAT-RW GNN message-passing kernel for 8 trn2 NeuronCores (Bass/Tile).

Strategy:
- Shard nodes across 8 cores (6250 real + 22 pad = 6272/core, 50176 padded).
- Per layer: each core builds its table shard [6272, 144] = [h(128) | ar(1) |
  pad(15)] from its own inputs, AllGather -> full table [50176, 144] in each
  core's DRAM, then gathers its 100352 walk endpoints (4 hops x 6272 x 16)
  row-by-row-per-partition via indirect DMA ([128,1] offsets -> [128,144]
  rows, the only gather primitive that works on this runtime).
- Node-major gather layout [128 nodes, 16 samples, 144]: softmax over samples
  is pure per-partition free-dim math; weighted mean = one fat strided DVE
  multiply + one strided reduce per (tile, hop).
- Head: PE transpose + lout matmul + log_softmax per 128-node tile.
"""
import sys
sys.path.insert(0, "/opt/trn_rl_repo")
sys.path.insert(0, "/root/.axon_site/_ro/trn_rl_repo")
import numpy as np

import bass_rust
import concourse.bass as bass
import concourse.mybir as mybir
import concourse.tile as tile
from concourse.bass_utils import run_bass_kernel_spmd

# problem constants (hardcoded per spec)
N, F_IN, H, C, K, NLAYER, RWS = 50000, 256, 128, 64, 4, 2, 16
NCORES = 8
NC_ = 6272                # padded nodes per core (49 * 128)
NPAD = NC_ * NCORES       # 50176
NT = NC_ // 128           # 49 node tiles per core
RW = 144                  # table row floats: h(128) | ar(1) | pad(15) -> 576B
P = 128
F32 = mybir.dt.float32
I32 = mybir.dt.int32


def _split_multi_waits(nc):
    """This walrus accepts at most one sem wait per instruction; Tile emits
    multi-wait instructions. Move extra waits onto same-engine carrier NOPs."""
    for f in nc.m.functions:
        for bb in f.blocks:
            insts = bb.instructions
            out = []
            changed = False
            for ins in insts:
                si = ins.sync_info
                ow = list(si.on_wait) if si is not None and si.on_wait else []
                keep = 0 if type(ins).__name__ == "InstDrain" else 1
                if len(ow) > keep:
                    move = ow[: len(ow) - keep]
                    stay = ow[len(ow) - keep:]
                    for w in move:
                        nop = mybir.InstNoOp(
                            name=nc.get_next_instruction_name(), ins=[], outs=[]
                        )
                        nop.engine = ins.engine
                        nop.sync_info = bass_rust.SyncInfo(on_wait=[w], on_update=[])
                        out.append(nop)
                    si.on_wait = stay
                    changed = True
                out.append(ins)
            if changed:
                bb.instructions = out


_QSEL = [None]
_ORIG_DMACOPY = mybir.InstDMACopy


class _QPatchedDMACopy:
    def __call__(self, **kw):
        if _QSEL[0] is not None and kw.get("queue") == "qPoolDynamic":
            kw["queue"] = _QSEL[0]
        return _ORIG_DMACOPY(**kw)


def _build_nc(att_np, phases=5):
    """Build the SPMD program (identical on all cores). att values are baked
    as immediates (available at kernel build time)."""
    mybir.InstDMACopy = _QPatchedDMACopy()
    nc = bass.Bass(num_swdge_queues=4)

    # per-core external inputs
    xTa = nc.dram_tensor("xTa", [P, NC_], F32, kind="ExternalInput")   # xT rows 0:128
    xTb = nc.dram_tensor("xTb", [P, NC_], F32, kind="ExternalInput")   # xT rows 128:256
    ends_g = nc.dram_tensor("ends_g", [NLAYER, P, K * NT * 16], I32, kind="ExternalInput")
    w0t = nc.dram_tensor("w0t", [2, P, H], F32, kind="ExternalInput")  # lin0_w.T k-chunks
    w1t = nc.dram_tensor("w1t", [P, H], F32, kind="ExternalInput")     # lin1_w.T
    loutt = nc.dram_tensor("loutt", [P, C], F32, kind="ExternalInput") # lout_w.T
    b0r = nc.dram_tensor("b0r", [P, H], F32, kind="ExternalInput")     # bias rows replicated
    b1r = nc.dram_tensor("b1r", [P, H], F32, kind="ExternalInput")
    bor = nc.dram_tensor("bor", [P, C], F32, kind="ExternalInput")
    attlr = nc.dram_tensor("attlr", [NLAYER, P, H], F32, kind="ExternalInput")
    ident_in = nc.dram_tensor("ident_in", [P, P], F32, kind="ExternalInput")
    attrr = nc.dram_tensor("attrr", [NLAYER, P, H], F32, kind="ExternalInput")

    out_sh = nc.dram_tensor("out_sh", [NC_, C], F32, kind="ExternalOutput")

    # internal DRAM
    t_shard = [nc.dram_tensor(f"t{l}_shard", [NC_, RW], F32) for l in range(NLAYER)]
    t_full = [
        nc.dram_tensor(f"t{l}_full", [NPAD, RW], F32, addr_space="Shared")
        for l in range(NLAYER)
    ]

    core_ids = list(range(NCORES))

    with tile.TileContext(nc) as tc:
        with (
            tc.tile_pool(name="const", bufs=1) as cpool,
            tc.tile_pool(name="bld", bufs=3) as bpool,
            tc.tile_pool(name="gat", bufs=3) as gpool,
            tc.tile_pool(name="mac", bufs=2) as mpool,
            tc.tile_pool(name="sm", bufs=4) as spool,
            tc.tile_pool(name="big", bufs=1) as bigpool,
            tc.tile_pool(name="ps", bufs=2, space="PSUM") as pspool,
            tc.tile_pool(name="ps2", bufs=2, space="PSUM") as ps2pool,
        ):
            # constants in SBUF
            ident = cpool.tile([P, P], F32)
            nc.sync.dma_start(out=ident[:], in_=ident_in[:])
            w0a_sb = cpool.tile([P, H], F32)
            nc.sync.dma_start(out=w0a_sb[:], in_=w0t[0])
            w0b_sb = cpool.tile([P, H], F32)
            nc.sync.dma_start(out=w0b_sb[:], in_=w0t[1])
            w1t_sb = cpool.tile([P, H], F32)
            nc.sync.dma_start(out=w1t_sb[:], in_=w1t[:])
            loutt_sb = cpool.tile([P, C], F32)
            nc.sync.dma_start(out=loutt_sb[:], in_=loutt[:])
            b0_sb = cpool.tile([P, H], F32)
            nc.sync.dma_start(out=b0_sb[:], in_=b0r[:])
            b1_sb = cpool.tile([P, H], F32)
            nc.sync.dma_start(out=b1_sb[:], in_=b1r[:])
            bo_sb = cpool.tile([P, C], F32)
            nc.sync.dma_start(out=bo_sb[:], in_=bor[:])
            attl_sb = [cpool.tile([P, H], F32, tag=f"attl{l}", name=f"attl{l}") for l in range(NLAYER)]
            attr_sb = [cpool.tile([P, H], F32, tag=f"attr{l}", name=f"attr{l}") for l in range(NLAYER)]
            for l in range(NLAYER):
                nc.sync.dma_start(out=attl_sb[l][:], in_=attlr[l])
                nc.sync.dma_start(out=attr_sb[l][:], in_=attrr[l])
            al_own = cpool.tile([P, NT], F32)
            agg_all = bigpool.tile([P, NC_], F32)   # agg tiles, col-blocked
            ends_sb = cpool.tile([P, K * NT * 16], I32)

            xa_sb = cpool.tile([P, NC_], F32)
            nc.sync.dma_start(out=xa_sb[:], in_=xTa[:])
            xb_sb = cpool.tile([P, NC_], F32)
            nc.sync.dma_start(out=xb_sb[:], in_=xTb[:])

            def build_tile_l0(t):
                ps = pspool.tile([P, H], F32, tag="bps")
                nc.tensor.matmul(out=ps[:], lhsT=xa_sb[:, t * P:(t + 1) * P],
                                 rhs=w0a_sb[:], start=True, stop=False)
                nc.tensor.matmul(out=ps[:], lhsT=xb_sb[:, t * P:(t + 1) * P],
                                 rhs=w0b_sb[:], start=False, stop=True)
                tb = bpool.tile([P, RW], F32, tag="tbl")
                nc.vector.tensor_tensor(out=tb[:, 0:H], in0=ps[:], in1=b0_sb[:],
                                        op=mybir.AluOpType.add)
                nc.vector.memset(tb[:, H:RW], 0.0)
                tr = mpool.tile([P, H], F32, tag="trash")
                nc.vector.tensor_tensor(out=tr[:], in0=tb[:, 0:H],
                                        in1=attr_sb[0][:], op=mybir.AluOpType.mult)
                nc.vector.tensor_reduce(out=tb[:, H:H + 1], in_=tr[:],
                                        axis=mybir.AxisListType.X,
                                        op=mybir.AluOpType.add)
                nc.vector.tensor_tensor(out=tr[:], in0=tb[:, 0:H],
                                        in1=attl_sb[0][:], op=mybir.AluOpType.mult)
                nc.vector.tensor_reduce(out=al_own[:, t:t + 1], in_=tr[:],
                                        axis=mybir.AxisListType.X,
                                        op=mybir.AluOpType.add)
                nc.sync.dma_start(out=t_shard[0][t * P:(t + 1) * P, :], in_=tb[:])

            def build_tile_l1(t):
                # transpose agg tile -> lhsT
                pst = ps2pool.tile([P, P], F32, tag="tps")
                nc.tensor.transpose(out=pst[:], in_=agg_all[:, t * P:(t + 1) * P],
                                    identity=ident[:])
                aggT = bpool.tile([P, P], F32, tag="aggT")
                nc.vector.tensor_copy(out=aggT[:], in_=pst[:])
                ps = pspool.tile([P, H], F32, tag="bps")
                nc.tensor.matmul(out=ps[:], lhsT=aggT[:], rhs=w1t_sb[:],
                                 start=True, stop=True)
                tb = bpool.tile([P, RW], F32, tag="tbl")
                nc.vector.tensor_tensor(out=tb[:, 0:H], in0=ps[:], in1=b1_sb[:],
                                        op=mybir.AluOpType.add)
                nc.vector.memset(tb[:, H:RW], 0.0)
                tr = mpool.tile([P, H], F32, tag="trash")
                nc.vector.tensor_tensor(out=tr[:], in0=tb[:, 0:H],
                                        in1=attr_sb[1][:], op=mybir.AluOpType.mult)
                nc.vector.tensor_reduce(out=tb[:, H:H + 1], in_=tr[:],
                                        axis=mybir.AxisListType.X,
                                        op=mybir.AluOpType.add)
                nc.vector.tensor_tensor(out=tr[:], in0=tb[:, 0:H],
                                        in1=attl_sb[1][:], op=mybir.AluOpType.mult)
                nc.vector.tensor_reduce(out=al_own[:, t:t + 1], in_=tr[:],
                                        axis=mybir.AxisListType.X,
                                        op=mybir.AluOpType.add)
                nc.sync.dma_start(out=t_shard[1][t * P:(t + 1) * P, :], in_=tb[:])

            def aggregate_layer(l):
                att0 = float(att_np[l, 0])
                nc.sync.dma_start(out=ends_sb[:], in_=ends_g[l])
                for t in range(NT):
                    agg_sl = agg_all[:, t * P:(t + 1) * P]
                    hown = gpool.tile([P, H], F32, tag="hown")
                    nc.sync.dma_start(out=hown[:],
                                      in_=t_shard[l][t * P:(t + 1) * P, 0:H])
                    nc.scalar.activation(out=agg_sl, in_=hown[:],
                                         func=mybir.ActivationFunctionType.Copy,
                                         scale=att0)
                    for k in range(K):
                        sc = float(att_np[l, k + 1]) / RWS
                        G = gpool.tile([P, RWS, RW], F32, tag="G")
                        base = k * NT * 16 + t * 16
                        for s in range(RWS):
                            _QSEL[0] = f"qPoolDynamic{(s % 4) or ''}"
                            nc.gpsimd.indirect_dma_start(
                                out=G[:, s, :], out_offset=None,
                                in_=t_full[l][:],
                                in_offset=bass.IndirectOffsetOnAxis(
                                    ap=ends_sb[:, base + s:base + s + 1], axis=0))
                            _QSEL[0] = None
                        ar_g = G[:, :, H]                      # [128, 16] strided
                        lg = spool.tile([P, RWS], F32, tag="lg")
                        nc.vector.tensor_scalar(
                            out=lg[:], in0=ar_g, scalar1=al_own[:, t:t + 1],
                            scalar2=None, op0=mybir.AluOpType.add)
                        lr = spool.tile([P, RWS], F32, tag="lr")
                        nc.scalar.activation(out=lr[:], in_=lg[:],
                                             func=mybir.ActivationFunctionType.Lrelu,
                                             alpha=0.2)
                        ex = spool.tile([P, RWS], F32, tag="ex")
                        nc.scalar.activation(out=ex[:], in_=lr[:],
                                             func=mybir.ActivationFunctionType.Exp)
                        sm = spool.tile([P, 1], F32, tag="sm")
                        nc.vector.tensor_reduce(out=sm[:], in_=ex[:],
                                                axis=mybir.AxisListType.X,
                                                op=mybir.AluOpType.add)
                        rc = spool.tile([P, 1], F32, tag="rc")
                        nc.vector.reciprocal(out=rc[:], in_=sm[:])
                        wt = spool.tile([P, RWS], F32, tag="wt")
                        nc.vector.tensor_scalar(
                            out=wt[:], in0=ex[:], scalar1=rc[:, 0:1], scalar2=sc,
                            op0=mybir.AluOpType.mult, op1=mybir.AluOpType.mult)
                        mt = mpool.tile([P, RWS, H], F32, tag="mt")
                        nc.vector.tensor_tensor(
                            out=mt[:], in0=G[:, :, 0:H],
                            in1=wt[:, :, None].broadcast_to([P, RWS, H]),
                            op=mybir.AluOpType.mult)
                        ht = mpool.tile([P, H], F32, tag="ht")
                        nc.vector.tensor_reduce(
                            out=ht[:], in_=mt[:].rearrange("p c f -> p f c"),
                            axis=mybir.AxisListType.X, op=mybir.AluOpType.add)
                        nc.vector.tensor_tensor(out=agg_sl, in0=agg_sl, in1=ht[:],
                                                op=mybir.AluOpType.add)

            def head_tile(t):
                pst = ps2pool.tile([P, P], F32, tag="tps")
                nc.tensor.transpose(out=pst[:], in_=agg_all[:, t * P:(t + 1) * P],
                                    identity=ident[:])
                aggT = bpool.tile([P, P], F32, tag="aggT")
                nc.vector.tensor_copy(out=aggT[:], in_=pst[:])
                ps = pspool.tile([P, C], F32, tag="hps")
                nc.tensor.matmul(out=ps[:], lhsT=aggT[:], rhs=loutt_sb[:],
                                 start=True, stop=True)
                lgt = bpool.tile([P, C], F32, tag="lgt")
                nc.vector.tensor_tensor(out=lgt[:], in0=ps[:], in1=bo_sb[:],
                                        op=mybir.AluOpType.add)
                mx = spool.tile([P, 1], F32, tag="mx")
                nc.vector.tensor_reduce(out=mx[:], in_=lgt[:],
                                        axis=mybir.AxisListType.X,
                                        op=mybir.AluOpType.max)
                mneg = spool.tile([P, 1], F32, tag="mneg")
                nc.vector.tensor_scalar(out=mneg[:], in0=mx[:], scalar1=-1.0,
                                        scalar2=None, op0=mybir.AluOpType.mult)
                ev = bpool.tile([P, C], F32, tag="ev")
                se = spool.tile([P, 1], F32, tag="se")
                nc.scalar.activation(out=ev[:], in_=lgt[:],
                                     func=mybir.ActivationFunctionType.Exp,
                                     bias=mneg[:, 0:1], accum_out=se[:])
                lnse = spool.tile([P, 1], F32, tag="lnse")
                nc.scalar.activation(out=lnse[:], in_=se[:],
                                     func=mybir.ActivationFunctionType.Ln)
                tot = spool.tile([P, 1], F32, tag="tot")
                nc.vector.tensor_tensor(out=tot[:], in0=mx[:], in1=lnse[:],
                                        op=mybir.AluOpType.add)
                ot = bpool.tile([P, C], F32, tag="ot")
                nc.vector.tensor_scalar(out=ot[:], in0=lgt[:], scalar1=tot[:, 0:1],
                                        scalar2=None,
                                        op0=mybir.AluOpType.subtract)
                nc.sync.dma_start(out=out_sh[t * P:(t + 1) * P, :], in_=ot[:])

            # ---- phases ----
            for t in range(NT):
                build_tile_l0(t)
            if phases >= 1:
                tc.strict_bb_all_engine_barrier()
                nc.gpsimd.collective_compute(
                    "AllGather", mybir.AluOpType.bypass,
                    replica_groups=[core_ids],
                    ins=[t_shard[0][:]], outs=[t_full[0][:]])
                tc.strict_bb_all_engine_barrier()
            if phases >= 2:
                aggregate_layer(0)
            if phases >= 3:
                for t in range(NT):
                    build_tile_l1(t)
                tc.strict_bb_all_engine_barrier()
                nc.gpsimd.collective_compute(
                    "AllGather", mybir.AluOpType.bypass,
                    replica_groups=[core_ids],
                    ins=[t_shard[1][:]], outs=[t_full[1][:]])
                tc.strict_bb_all_engine_barrier()
            if phases >= 4:
                aggregate_layer(1)
            if phases >= 5:
                for t in range(NT):
                    head_tile(t)
            if phases < 5:
                z = bpool.tile([P, C], F32, tag="z")
                nc.vector.memset(z[:], 0.0)
                for t in range(NT):
                    nc.sync.dma_start(out=out_sh[t * P:(t + 1) * P, :], in_=z[:])

    mybir.InstDMACopy = _ORIG_DMACOPY
    from concourse.library_overlay import lower_extended_insts
    lower_extended_insts(nc)
    _split_multi_waits(nc)
    return nc


def _prep_inputs(x, ends, lin0_w, lin0_b, lin1_w, lin1_b, lout_w, lout_b,
                 attl_w, attl_b, attr_w, attr_b, att):
    ends = np.asarray(ends).astype(np.int32)
    x = np.asarray(x, dtype=np.float32)
    # pad + transpose x
    xp = np.zeros((NPAD, F_IN), np.float32)
    # node remap: global g -> padded core-local layout
    # core = g // 6250, local = g % 6250, pid = core*6272 + local
    g = np.arange(N)
    pid_of = (g // 6250) * NC_ + (g % 6250)
    xp[pid_of] = x
    xT = np.ascontiguousarray(xp.T)                      # [256, 50176]

    # remap endpoint ids to padded space
    er = ends.reshape(NLAYER, K, N, RWS)
    er_pid = pid_of[er]                                  # [L, K, N, RWS]

    in_maps = []
    for c in range(NCORES):
        lo, hi = c * 6250, (c + 1) * 6250
        # per-core ends_g [L, 128, K*NT*16]: node tile t partition p ->
        # node local = t*128+p (pad nodes -> endpoint 0)
        eg = np.zeros((NLAYER, P, K * NT * 16), np.int32)
        for l in range(NLAYER):
            src = er_pid[l, :, lo:hi, :]                 # [K, 6250, 16]
            full = np.zeros((K, NC_, RWS), np.int32)
            full[:, :6250, :] = src
            # layout col = k*NT*16 + t*16 + s ; partition = p
            v = full.reshape(K, NT, P, RWS)
            eg[l] = v.transpose(2, 0, 1, 3).reshape(P, K * NT * RWS)
        in_maps.append({
            "xTa": np.ascontiguousarray(xT[:128, c * NC_:(c + 1) * NC_]),
            "xTb": np.ascontiguousarray(xT[128:, c * NC_:(c + 1) * NC_]),
            "ends_g": eg,
            "w0t": np.ascontiguousarray(
                lin0_w.T.astype(np.float32).reshape(2, P, H)),
            "w1t": np.ascontiguousarray(lin1_w.T.astype(np.float32)),
            "loutt": np.ascontiguousarray(lout_w.T.astype(np.float32)),
            "b0r": np.tile(np.asarray(lin0_b, np.float32), (P, 1)),
            "b1r": np.tile(np.asarray(lin1_b, np.float32), (P, 1)),
            "bor": np.tile(np.asarray(lout_b, np.float32), (P, 1)),
            "ident_in": np.eye(P, dtype=np.float32),
            "attlr": np.stack([np.tile(np.asarray(attl_w, np.float32)[l], (P, 1))
                               for l in range(NLAYER)]),
            "attrr": np.stack([np.tile(np.asarray(attr_w, np.float32)[l], (P, 1))
                               for l in range(NLAYER)]),
        })
    return in_maps, pid_of


_CACHE = {}


def kernel(**inputs):
    att = np.asarray(inputs["att"], dtype=np.float32)
    key = att.tobytes()
    if key not in _CACHE:
        _CACHE[key] = _build_nc(att)
    nc = _CACHE[key]
    in_maps, pid_of = _prep_inputs(**inputs)
    res = run_bass_kernel_spmd(nc, in_maps, list(range(NCORES)))
    out = np.zeros((N, C), np.float32)
    for c in range(NCORES):
        out[c * 6250:(c + 1) * 6250] = res.results[c]["out_sh"][:6250]
    return out
